# revision 23
# baseline (speedup 1.0000x reference)
"""DenseEdgeGAT layer on 8 trn2 NeuronCores (Bass/Tile).

Strategy (row-sharding over target nodes n, 128 rows per core):
  - Every core computes LN(x) + k/v for all 1024 source nodes (replicated,
    cheap) and q for its own 128-row slab.
  - edge_feat slab is uploaded twice in 2-byte form with different layouts:
      * n-major fp16  -> xbar DMA-transpose gives X[(m%8,e), j, n] feeding the
        block-diagonal Wae matmuls (edge attention bias).
      * m-major bf16  -> plain reshaping DMA gives Z[m%128, mc, n, e] feeding
        the per-node T-stage (edge-value contraction), using
        T[n,h,e] = sum_m p[n,m,h] ef[n,m,e]  and  out2 = T @ Wve(blockdiag).
  - softmax uses a fixed shift M=50 (shift-invariant, no rowmax pass); the
    mask enters as additive -10000*(1-mask) folded into the bias tensor, and
    bae - M is folded into the score merge. exp runs on the scalar engine
    with fused row-sum accumulation (denominators); 1/denom is applied once
    at the end (flash-attention style).
"""

import numpy as np
import ml_dtypes

import concourse.bacc as bacc
import concourse.bass as bass
import concourse.tile as tile
from concourse import mybir
from concourse.bass_utils import run_bass_kernel_spmd
from concourse.masks import make_identity

F32 = mybir.dt.float32
F32R = mybir.dt.float32r
F16 = mybir.dt.float16
BF16 = mybir.dt.bfloat16

N = 1024
DIM = 128
H = 8
DK = 16
E = 16
P = 128
NCORES = 8
R = N // NCORES  # 128 rows per core
NMC = N // P     # 8 m-chunks
SHIFT = 50.0
EPS = 1e-5
NEG = -10000.0

AF = mybir.ActivationFunctionType
OP = mybir.AluOpType

_CACHED = None


def _build_program(dbg=False):
    nc = bacc.Bacc("TRN2", target_bir_lowering=False, debug=False,
                   num_devices=NCORES)

    dram = {}

    def din(name, shape, dt):
        dram[name] = nc.dram_tensor(name, shape, dt, kind="ExternalInput").ap()
        return dram[name]

    efn_d = din("efn", [R, N * E], F16)        # n-major slab [n, (m, e)]
    efm_d = din("efm", [N, R * E], BF16)       # m-major slab [m, (n, e)]
    mneg_d = din("mneg", [R, N], F32)          # (mask-1)*10000
    x_d = din("x", [N, DIM], F32)
    xs_d = din("xs", [R, DIM], F32)            # this core's slab of x
    wq_d = din("wq", [DIM, 3 * DIM], F32)   # head-padded: plane s=h//3, group g=h%3
    wk_d = din("wk", [DIM, 3 * DIM], F32)
    wv_d = din("wv", [DIM, DIM], F32)
    wo_d = din("wo", [DIM, DIM], F32)
    wblk_d = din("wblk", [P, 8 * H], F16)      # block-diag Wae
    wve_d = din("wve", [E, DIM], F32)
    bq_d = din("bq", [DIM, 3], F32)          # head-padded per plane
    bk_d = din("bk", [DIM, 3], F32)
    bvrep_d = din("bvrep", [P, DIM], F32)
    borep_d = din("borep", [P, DIM], F32)
    grep_d = din("grep", [P, DIM], F32)
    brep_d = din("brep", [P, DIM], F32)
    baerep_d = din("baerep", [P, H], F32)      # bae - SHIFT, replicated

    out_d = nc.dram_tensor("out", [R, DIM], F32, kind="ExternalOutput").ap()

    with tile.TileContext(nc) as tc:
        with tc.tile_pool(name="consts", bufs=1) as consts, \
             tc.tile_pool(name="big", bufs=1) as big, \
             tc.tile_pool(name="xring", bufs=2) as xring, \
             tc.tile_pool(name="attring", bufs=2) as attring, \
             tc.tile_pool(name="work", bufs=3) as work, \
             tc.tile_pool(name="psbig", bufs=2, space="PSUM") as psbig, \
             tc.tile_pool(name="pssm", bufs=2, space="PSUM") as pssm:

            # ---------- constants / params ----------
            ident = consts.tile([P, P], F32)
            make_identity(nc, ident)
            identb = consts.tile([P, P], BF16)
            make_identity(nc, identb)

            wblk_t = consts.tile([P, 8 * H], F16)
            nc.sync.dma_start(out=wblk_t, in_=wblk_d)
            mneg_t = consts.tile([R, N], F32)
            nc.sync.dma_start(out=mneg_t, in_=mneg_d)
            baerep_t = consts.tile([P, H], F32)
            nc.sync.dma_start(out=baerep_t, in_=baerep_d)
            bq_t = consts.tile([DIM, 3], F32)
            nc.sync.dma_start(out=bq_t, in_=bq_d)
            bk_t = consts.tile([DIM, 3], F32)
            nc.sync.dma_start(out=bk_t, in_=bk_d)
            bvrep_t = consts.tile([P, DIM], F32)
            nc.sync.dma_start(out=bvrep_t, in_=bvrep_d)
            borep_t = consts.tile([P, DIM], F32)
            nc.sync.dma_start(out=borep_t, in_=borep_d)
            grep_t = consts.tile([P, DIM], F32)
            nc.sync.dma_start(out=grep_t, in_=grep_d)
            brep_t = consts.tile([P, DIM], F32)
            nc.sync.dma_start(out=brep_t, in_=brep_d)

            # params that feed float32r matmuls: round via DVE copy
            def round_param(name, d_ap, shape):
                raw = work.tile(shape, F32, tag="praw")
                nc.sync.dma_start(out=raw, in_=d_ap)
                r = consts.tile(shape, F32R, tag=name)
                nc.vector.tensor_copy(r, raw)
                return r

            wq_r = round_param("wqr", wq_d, [DIM, 3 * DIM])
            wk_r = round_param("wkr", wk_d, [DIM, 3 * DIM])
            wv_r = round_param("wvr", wv_d, [DIM, DIM])
            wo_r = round_param("wor", wo_d, [DIM, DIM])
            wve_r = round_param("wver", wve_d, [E, DIM])

            # ---------- big persistent tensors ----------
            z_sb = big.tile([P, NMC, R * E], BF16)     # Z[mm, mc, n, e]
            nc.sync.dma_start(out=z_sb, in_=efm_d.rearrange("(a b) c -> b a c", b=P))

            bias_sb = big.tile([R, N * H], F32)        # [n, (j, c, h)] raw bias+maskneg
            p_sb = big.tile([R, H, N], BF16)           # exp'd attention [n, h, m]
            pt_sb = big.tile([P, H, NMC, R], BF16)     # p transposed [mm, h, mc, n]
            kt_sb = big.tile([DIM, 3, N], F32R)        # k^T head-padded planes
            qts_sb = big.tile([DIM, 3, R], F32R)       # q^T slab, padded, pre-scaled
            ht_sb = big.tile([DIM, N], F32R)           # h^T [din, m]
            v_sb = big.tile([P, NMC, DIM], BF16)       # v [mm, mc, dout]
            den_t = big.tile([R, H], F32)              # softmax denominators

            # ---------- LN over all nodes (+ slab as chunk 8) ----------
            xall = big.tile([P, NMC + 1, DIM], F32)
            nc.sync.dma_start(out=xall[:, 0:NMC, :],
                              in_=x_d.rearrange("(g b) d -> b g d", b=P))
            nc.sync.dma_start(out=xall[:, NMC, :], in_=xs_d)

            s1 = work.tile([P, NMC + 1], F32, tag="lnstat")
            nc.vector.tensor_reduce(out=s1, in_=xall, axis=mybir.AxisListType.X,
                                    op=OP.add)
            mu = work.tile([P, NMC + 1], F32, tag="lnstat")
            nc.vector.tensor_scalar_mul(mu, s1, 1.0 / DIM)
            xsq = work.tile([P, NMC + 1, DIM], F32, tag="lnbig")
            nc.vector.tensor_tensor(out=xsq, in0=xall, in1=xall, op=OP.mult)
            s2 = work.tile([P, NMC + 1], F32, tag="lnstat")
            nc.vector.tensor_reduce(out=s2, in_=xsq, axis=mybir.AxisListType.X,
                                    op=OP.add)
            ex2 = work.tile([P, NMC + 1], F32, tag="lnstat")
            nc.vector.tensor_scalar_mul(ex2, s2, 1.0 / DIM)
            mu2 = work.tile([P, NMC + 1], F32, tag="lnstat")
            nc.vector.tensor_tensor(out=mu2, in0=mu, in1=mu, op=OP.mult)
            var = work.tile([P, NMC + 1], F32, tag="lnstat")
            nc.vector.tensor_tensor(out=var, in0=ex2, in1=mu2, op=OP.subtract)
            eps_t = consts.tile([P, 1], F32)
            nc.vector.memset(eps_t, EPS)
            sd = work.tile([P, NMC + 1], F32, tag="lnstat")
            nc.scalar.activation(out=sd, in_=var, func=AF.Sqrt, bias=eps_t)
            rstd = work.tile([P, NMC + 1], F32, tag="lnstat")
            nc.vector.reciprocal(rstd, sd)

            hall = big.tile([P, NMC + 1, DIM], F32)
            xc = work.tile([P, NMC + 1, DIM], F32, tag="lnbig")
            nc.vector.tensor_tensor(
                out=xc, in0=xall,
                in1=mu.unsqueeze(2).broadcast_to([P, NMC + 1, DIM]),
                op=OP.subtract)
            nc.vector.tensor_tensor(
                out=xc, in0=xc,
                in1=rstd.unsqueeze(2).broadcast_to([P, NMC + 1, DIM]),
                op=OP.mult)
            nc.gpsimd.tensor_tensor(
                out=hall, in0=xc,
                in1=grep_t.unsqueeze(1).broadcast_to([P, NMC + 1, DIM]),
                op=OP.mult)
            nc.gpsimd.tensor_tensor(
                out=hall, in0=hall,
                in1=brep_t.unsqueeze(1).broadcast_to([P, NMC + 1, DIM]),
                op=OP.add)

            # ---------- h^T via PE transposes ----------
            hst_sb = big.tile([DIM, R], F32R)  # h^T of the slab
            for g in range(NMC + 1):
                tp = pssm.tile([P, P], F32, tag="ps")
                nc.tensor.transpose(tp, hall[:, g, :], ident)
                if g < NMC:
                    nc.vector.tensor_copy(ht_sb[:, g * P:(g + 1) * P], tp)
                else:
                    nc.vector.tensor_copy(hst_sb, tp)

            # ---------- projections ----------
            # k^T = Wk^T-form: lhsT=Wk_pad plane, rhs=h^T [din, m]
            for s in range(3):
                for i in range(2):
                    kp = psbig.tile([DIM, 512], F32, tag="pb")
                    nc.tensor.matmul(kp, wk_r[:, s * DIM:(s + 1) * DIM],
                                     ht_sb[:, i * 512:(i + 1) * 512],
                                     start=True, stop=True)
                    nc.vector.tensor_scalar(
                        out=kt_sb[:, s, i * 512:(i + 1) * 512],
                        in0=kp, scalar1=bk_t[:, s:s + 1], scalar2=None,
                        op0=OP.add)
                qp = pssm.tile([DIM, R], F32, tag="ps")
                nc.tensor.matmul(qp, wq_r[:, s * DIM:(s + 1) * DIM], hst_sb,
                                 start=True, stop=True)
                nc.vector.tensor_scalar(out=qts_sb[:, s, :], in0=qp,
                                        scalar1=bq_t[:, s:s + 1],
                                        scalar2=0.25, op0=OP.add, op1=OP.mult)
            # v natural: per chunk, lhsT=h^T chunk [din, m128], rhs=Wv
            for mc in range(NMC):
                vp = pssm.tile([P, DIM], F32, tag="ps")
                nc.tensor.matmul(vp, ht_sb[:, mc * P:(mc + 1) * P], wv_r,
                                 start=True, stop=True)
                nc.vector.scalar_tensor_tensor(
                    out=v_sb[:, mc, :], in0=vp, scalar=1.0, in1=bvrep_t,
                    op0=OP.mult, op1=OP.add)

            # ---------- edge bias: X via one xbar + block-diag matmuls ------
            x_full = big.tile([P, 128, P], F16)    # X[(c,e), j, n]
            nc.sync.dma_start(out=x_full, in_=efn_d, transpose=True)
            for mc in range(NMC):
                b_ps = psbig.tile([R, 16 * 8 * H], F32, tag="pb")
                for xloc in range(16):
                    j = mc * 16 + xloc
                    nc.tensor.matmul(b_ps[:, xloc * 64:(xloc + 1) * 64],
                                     x_full[:, j, :], wblk_t,
                                     start=True, stop=True)
                # bias_sb[n, (x, c, h)] = b_ps + mneg broadcast-over-h
                msl = mneg_t[:, mc * P:(mc + 1) * P] \
                    .rearrange("p (x c) -> p x c", c=8) \
                    .unsqueeze(3).broadcast_to([R, 16, 8, H])
                nc.vector.scalar_tensor_tensor(
                    out=bias_sb[:, mc * 1024:(mc + 1) * 1024]
                        .rearrange("p (x c h) -> p x c h", c=8, h=H),
                    in0=b_ps.rearrange("p (x c h) -> p x c h", c=8, h=H),
                    scalar=1.0, in1=msl, op0=OP.mult, op1=OP.add)

            # ---------- scores + merge + exp per head ----------
            bias_v = bias_sb.rearrange("p (j c h) -> p j c h", c=8, h=H)
            for h in range(H):
                s, g = h // 3, h % 3
                sc_ps = psbig.tile([R, N], F32, tag="pb")
                for i in range(2):
                    nc.tensor.matmul(sc_ps[:, i * 512:(i + 1) * 512],
                                     qts_sb[g * 32:(g + 1) * 32, s, :],
                                     kt_sb[g * 32:(g + 1) * 32, s,
                                           i * 512:(i + 1) * 512],
                                     start=True, stop=True)
                att_t = attring.tile([R, N], F32)
                nc.vector.scalar_tensor_tensor(
                    out=att_t.rearrange("p (j c) -> p j c", c=8),
                    in0=sc_ps.rearrange("p (j c) -> p j c", c=8),
                    scalar=baerep_t[:, h:h + 1],
                    in1=bias_v[:, :, :, h],
                    op0=OP.add, op1=OP.add)
                nc.scalar.activation(out=p_sb[:, h, :], in_=att_t, func=AF.Exp,
                                     accum_out=den_t[:, h:h + 1])

            # ---------- p^T via PE transposes ----------
            for mc in range(NMC):
                for h in range(H):
                    ptp = pssm.tile([P, P], BF16, tag="ps")
                    nc.tensor.transpose(ptp, p_sb[:, h, mc * P:(mc + 1) * P],
                                        identb)
                    nc.vector.tensor_copy(pt_sb[:, h, mc, :], ptp)

            # ---------- T-stage: per (mc, n) matmuls -> Tt [e, (n, h)] ------
            t_ps = psbig.tile([E, R * H], F32, tag="pb")
            for n in range(R):
                for mc in range(NMC):
                    nc.tensor.matmul(
                        t_ps[:, n * H:(n + 1) * H],
                        z_sb[:, mc, n * E:(n + 1) * E],
                        pt_sb[:, :, mc, n],
                        start=(mc == 0), stop=(mc == NMC - 1))
            tt_sb = big.tile([E, R * H], F32R)
            nc.vector.tensor_copy(tt_sb, t_ps)
            tt_v = tt_sb.rearrange("p (n h) -> p n h", h=H)

            # ---------- out1 + out2 accumulation [n, (h, d)] ----------------
            o_ps = pssm.tile([R, DIM], F32, tag="ps")
            for h in range(H):
                for mc in range(NMC):
                    nc.tensor.matmul(o_ps[:, h * DK:(h + 1) * DK],
                                     pt_sb[:, h, mc, :],
                                     v_sb[:, mc, h * DK:(h + 1) * DK],
                                     start=(mc == 0), stop=False)
                nc.tensor.matmul(o_ps[:, h * DK:(h + 1) * DK],
                                 tt_v[:, :, h],
                                 wve_r[:, h * DK:(h + 1) * DK],
                                 start=False, stop=True)

            rden = work.tile([R, H], F32, tag="rden")
            nc.vector.reciprocal(rden, den_t)
            attn_sb = work.tile([R, DIM], F32, tag="attn")
            nc.vector.scalar_tensor_tensor(
                out=attn_sb.rearrange("p (h d) -> p h d", h=H),
                in0=o_ps.rearrange("p (h d) -> p h d", h=H),
                scalar=1.0,
                in1=rden.unsqueeze(2).broadcast_to([R, H, DK]),
                op0=OP.mult, op1=OP.mult)

            # ---------- output projection + residual ----------
            at_ps = pssm.tile([DIM, R], F32, tag="ps")
            nc.tensor.transpose(at_ps, attn_sb, ident)
            attnT = work.tile([DIM, R], F32R, tag="attnT")
            nc.vector.tensor_copy(attnT, at_ps)
            fo_ps = pssm.tile([R, DIM], F32, tag="ps")
            nc.tensor.matmul(fo_ps, attnT, wo_r, start=True, stop=True)

            out_sb = work.tile([R, DIM], F32, tag="osb")
            nc.vector.scalar_tensor_tensor(
                out=out_sb, in0=fo_ps, scalar=1.0, in1=borep_t,
                op0=OP.mult, op1=OP.add)
            nc.vector.tensor_tensor(out=out_sb, in0=out_sb,
                                    in1=xall[:, NMC, :], op=OP.add)
            nc.sync.dma_start(out=out_d, in_=out_sb)

            if dbg:
                def dout(name, tl):
                    d = nc.dram_tensor(name, list(tl.shape),
                                       tl.dtype, kind="ExternalOutput").ap()
                    nc.sync.dma_start(out=d, in_=tl)
                dout("d_ht", ht_sb)
                dout("d_kt", kt_sb)
                dout("d_qts", qts_sb)
                dout("d_v", v_sb)
                dout("d_bias", bias_sb)
                dout("d_p", p_sb)
                dout("d_den", den_t)
                dout("d_tt", tt_sb)
                dout("d_attn", attn_sb)
                dout("d_pt", pt_sb)
                dout("d_z", z_sb)

    nc.compile()
    return nc


def _get_program():
    global _CACHED
    if _CACHED is None:
        _CACHED = _build_program()
    return _CACHED


def _make_in_maps(inputs):
    x = np.ascontiguousarray(np.asarray(inputs["x"], dtype=np.float32))
    ef = np.asarray(inputs["edge_feat"], dtype=np.float32)
    mask = np.asarray(inputs["mask"])
    Wq = np.ascontiguousarray(np.asarray(inputs["Wq"], dtype=np.float32))
    Wk = np.ascontiguousarray(np.asarray(inputs["Wk"], dtype=np.float32))
    Wv = np.ascontiguousarray(np.asarray(inputs["Wv"], dtype=np.float32))
    Wo = np.ascontiguousarray(np.asarray(inputs["Wo"], dtype=np.float32))
    Wae = np.asarray(inputs["Wae"], dtype=np.float32)
    Wve = np.ascontiguousarray(np.asarray(inputs["Wve"], dtype=np.float32))
    bq = np.asarray(inputs["bq"], dtype=np.float32)
    bk = np.asarray(inputs["bk"], dtype=np.float32)
    bv = np.asarray(inputs["bv"], dtype=np.float32)
    bo = np.asarray(inputs["bo"], dtype=np.float32)
    bae = np.asarray(inputs["bae"], dtype=np.float32)
    gamma = np.asarray(inputs["gamma"], dtype=np.float32)
    beta = np.asarray(inputs["beta"], dtype=np.float32)

    # block-diagonal Wae: wblk[c*16+e, c*8+h] = Wae[e, h]
    wblk = np.zeros((P, 8 * H), dtype=np.float16)
    for c in range(8):
        wblk[c * E:(c + 1) * E, c * H:(c + 1) * H] = Wae.astype(np.float16)

    # head-padded projection weights/biases: head h -> plane s=h//4,
    # partition group g=h%4 (rows g*32..g*32+16), rest zero.
    def pad_w(W, b):
        Wp = np.zeros((DIM, 3 * DIM), dtype=np.float32)
        bp = np.zeros((DIM, 3), dtype=np.float32)
        for h in range(H):
            s, g = h // 3, h % 3
            Wp[:, s * DIM + g * 32:s * DIM + g * 32 + DK] = \
                W[:, h * DK:(h + 1) * DK]
            bp[g * 32:g * 32 + DK, s] = b[h * DK:(h + 1) * DK]
        return Wp, bp

    Wq_p, bq_p = pad_w(Wq, bq)
    Wk_p, bk_p = pad_w(Wk, bk)

    rep = lambda vec: np.ascontiguousarray(
        np.broadcast_to(vec.reshape(1, -1), (P, vec.size)).astype(np.float32))

    shared = {
        "x": x,
        "wq": Wq_p, "wk": Wk_p, "wv": Wv, "wo": Wo,
        "wblk": wblk, "wve": Wve,
        "bq": bq_p, "bk": bk_p,
        "bvrep": rep(bv), "borep": rep(bo),
        "grep": rep(gamma), "brep": rep(beta),
        "baerep": rep(bae - SHIFT),
    }

    in_maps = []
    for c in range(NCORES):
        rows = slice(c * R, (c + 1) * R)
        ef_slab = ef[rows]                                   # [R, N, E]
        efn = np.ascontiguousarray(ef_slab.reshape(R, N * E)).astype(np.float16)
        efm = np.ascontiguousarray(
            ef_slab.transpose(1, 0, 2).reshape(N, R * E)).astype(
                ml_dtypes.bfloat16)
        mneg = ((mask[rows].astype(np.float32) - 1.0) * (-NEG)).astype(
            np.float32)
        in_maps.append({
            **shared,
            "efn": efn, "efm": efm, "mneg": mneg,
            "xs": np.ascontiguousarray(x[rows]),
        })
    return in_maps


def kernel(**inputs) -> np.ndarray:
    in_maps = _make_in_maps(inputs)
    nc = _get_program()
    res = run_bass_kernel_spmd(nc, in_maps, list(range(NCORES)))
    out = np.concatenate([res.results[c]["out"] for c in range(NCORES)],
                         axis=0)
    return out.astype(np.float32)
